# revision 2
# baseline (speedup 1.0000x reference)
"""DenseCapsule dynamic-routing kernel for 8 Trainium2 NeuronCores.

Problem (per reference):
  x      [B=64, K=2048, Q=8]   fp32
  weight [J=32, K=2048, P=16, Q=8] fp32
  x_hat[b,j,k,p] = sum_q W[j,k,p,q] x[b,k,q]
  3 routing iterations (softmax over j, squash over p)
  out [B, J, P]

Sharding: data-parallel over batch (8 batches/core), weight replicated.

Per-core kernel strategy (memory regime):
  - x_hat (33.5MB fp32 for 8 batches) is NEVER written to HBM. It is
    computed once on the PE and kept SBUF-resident in bf16 (16.8MB).
  - Phase 1 matmul uses a block-diagonal x trick so the tiny q=8
    contraction still runs at full PE rate:
      lhsT = xbd[t]  [(k16,q8)=128, (k16,b8)=128]  (block-diag, host-built)
      rhs  = W_re[t] [(k16,q8)=128, (p16,j32)=512]
      out  = psum    [(k16,b8)=128, (p16,j32)=512] = x_hat tile
  - s[b,j,p] = sum_k c*x_hat runs on PE as a selector matmul: contract
    the (k16,b8) partition dim against a constant delta_b selector.
  - db[b,j,k] = sum_p v*x_hat runs on DVE: 4x-mode multiply then an
    in-place fold-tree over p (innermost j stays stride-1 = packed).
  - softmax over j: ACT exp + DVE reduces, j innermost.
"""

import numpy as np
import ml_dtypes

B, K, Q, J, P = 64, 2048, 8, 32, 16
NC_N = 8          # cores
BL = B // NC_N    # local batch = 8
KT = 16           # k's per tile
T = K // KT       # 128 tiles
N = P * J         # 512 free (p,j) layout: idx = p*32 + j
TC = 16           # k-tiles per routing chunk
NCH = T // TC     # 8 chunks

# weight dtype for the phase-1 matmul: "bf16" halves the dominant HBM
# stream (33.5MB -> 16.8MB); "fp32r" keeps fp32 inputs at full PE rate.
WDT = "bf16"

_CACHE = {}


def _prep(x, weight):
    x = np.ascontiguousarray(np.asarray(x, dtype=np.float32))
    weight = np.ascontiguousarray(np.asarray(weight, dtype=np.float32))
    np_wdt = ml_dtypes.bfloat16 if WDT == "bf16" else np.float32

    # W_re[t, ks*8+q, p*32+j] = W[j, t*16+ks, p, q]
    w5 = weight.reshape(J, T, KT, P, Q)
    w_re = np.ascontiguousarray(
        w5.transpose(1, 2, 4, 3, 0).reshape(T, KT * Q, N).astype(np_wdt)
    )

    # xbd per core: [t, ks*8+q, ks*8+b] = x[b, t*16+ks, q]
    xbds = []
    for c in range(NC_N):
        xc = x[c * BL : (c + 1) * BL]                      # [8, K, Q]
        xr = xc.reshape(BL, T, KT, Q).transpose(1, 2, 3, 0)  # [t, ks, q, b]
        z = np.zeros((T, KT, Q, KT, BL), dtype=np_wdt)
        for ks in range(KT):
            z[:, ks, :, ks, :] = xr[:, ks]
        xbds.append(z.reshape(T, KT * Q, KT * BL))

    # selector: sel8[ks*8+b, b'] = (b == b')
    sel = np.zeros((KT * BL, BL), dtype=ml_dtypes.bfloat16)
    for ks in range(KT):
        for b in range(BL):
            sel[ks * BL + b, b] = 1.0
    return w_re, xbds, sel


def _build_program():
    import concourse.bass as bass
    import concourse.tile as tile
    import concourse.mybir as mybir
    from concourse import bacc

    f32 = mybir.dt.float32
    bf16 = mybir.dt.bfloat16
    wdt = bf16 if WDT == "bf16" else mybir.dt.float32r
    alu = mybir.AluOpType
    act = mybir.ActivationFunctionType

    nc = bacc.Bacc("TRN2", target_bir_lowering=False, debug=False)

    w_d = nc.dram_tensor("w_re", [T, KT * Q, N], wdt, kind="ExternalInput")
    xbd_d = nc.dram_tensor("xbd", [T, KT * Q, KT * BL], wdt, kind="ExternalInput")
    sel_d = nc.dram_tensor("sel", [KT * BL, BL], bf16, kind="ExternalInput")
    out_d = nc.dram_tensor("out", [BL, N], f32, kind="ExternalOutput")

    with tile.TileContext(nc) as tc:
        with (
            tc.tile_pool(name="xhat", bufs=1) as xhat_pool,
            tc.tile_pool(name="wp", bufs=3) as wp,
            tc.tile_pool(name="xbp", bufs=3) as xbp,
            tc.tile_pool(name="selp", bufs=1) as selp,
            tc.tile_pool(name="mbuf", bufs=2) as mpool,
            tc.tile_pool(name="blogp", bufs=1) as blogp,
            tc.tile_pool(name="cbufp", bufs=1) as cbufp,
            tc.tile_pool(name="small", bufs=2) as small,
            tc.tile_pool(name="vrepp", bufs=2) as vrepp,
            tc.tile_pool(name="ph", bufs=4, space="PSUM") as ph_pool,
            tc.tile_pool(name="ps", bufs=3, space="PSUM") as ps_pool,
        ):
            # persistent tensors
            X = xhat_pool.tile([128, T * N], bf16)       # x_hat, bf16
            sel_sb = selp.tile([128, BL], bf16)
            nc.sync.dma_start(sel_sb[:], sel_d.ap())
            blog = blogp.tile([128, T * J], bf16)        # b_logits [(k,b),(t,j)]
            cbuf = cbufp.tile([128, T * J], bf16)        # exp / c   [(k,b),(t,j)]

            # ---------------- phase 1: x_hat + s0 ----------------
            s0_ps = ps_pool.tile([BL, N], f32, tag="s")
            for t in range(T):
                wt = wp.tile([128, N], wdt)
                nc.sync.dma_start(wt[:], w_d.ap()[t])
                xb = xbp.tile([128, KT * BL], wdt)
                nc.sync.dma_start(xb[:], xbd_d.ap()[t])
                ph = ph_pool.tile([128, N], f32)
                nc.tensor.matmul(ph[:], xb[:], wt[:], start=True, stop=True)
                # cast to bf16 into resident X (ACT engine; DVE is busy later)
                nc.scalar.copy(X[:, t * N : (t + 1) * N], ph[:])
                # s0 accumulation: contract (k16,b8) with delta_b selector
                nc.tensor.matmul(
                    s0_ps[:],
                    sel_sb[:],
                    X[:, t * N : (t + 1) * N],
                    start=(t == 0),
                    stop=(t == T - 1),
                )

            def squash(s_ps, scale, want_vrep):
                """v = squash(scale * s_ps) over p.  Returns (v fp32 [8,N],
                vrep bf16 [128,N] or None)."""
                s_s = small.tile([BL, N], f32, tag="s_s")
                nc.vector.tensor_scalar_mul(s_s[:], s_ps[:], scale)
                sq = small.tile([BL, N], f32, tag="sq")
                nc.vector.tensor_tensor(sq[:], s_s[:], s_s[:], alu.mult)
                # fold over p (p stride = J, j stride = 1)
                v3 = sq[:].rearrange("b (p j) -> b p j", p=P)
                for h in (8, 4, 2, 1):
                    nc.vector.tensor_tensor(
                        v3[:, 0:h, :], v3[:, 0:h, :], v3[:, h : 2 * h, :], alu.add
                    )
                n2 = small.tile([BL, J], f32, tag="n2")
                nc.vector.tensor_copy(n2[:], v3[:, 0, :])
                nrm = small.tile([BL, J], f32, tag="nrm")
                nc.scalar.sqrt(nrm[:], n2[:])
                den = small.tile([BL, J], f32, tag="den")
                nc.vector.tensor_scalar_add(den[:], n2[:], 1.0)
                rec = small.tile([BL, J], f32, tag="rec")
                nc.vector.reciprocal(rec[:], den[:])
                fct = small.tile([BL, J], f32, tag="fct")
                nc.vector.tensor_tensor(fct[:], nrm[:], rec[:], alu.mult)
                v = small.tile([BL, N], f32, tag="v")
                fb = fct[:].unsqueeze(1).broadcast_to([BL, P, J])
                nc.vector.tensor_tensor(
                    v[:].rearrange("b (p j) -> b p j", p=P),
                    s_s[:].rearrange("b (p j) -> b p j", p=P),
                    fb,
                    alu.mult,
                )
                vrep = None
                if want_vrep:
                    vbf = small.tile([BL, N], bf16, tag="vbf")
                    nc.vector.tensor_copy(vbf[:], v[:])
                    vrep = vrepp.tile([128, N], bf16)
                    for ks in range(KT):
                        nc.sync.dma_start(vrep[ks * BL : (ks + 1) * BL, :], vbf[:])
                return v, vrep

            def db_phase(vrep, first):
                """blog (+)= sum_p X * vrep, chunked over t."""
                for ch in range(NCH):
                    t0 = ch * TC
                    m = mpool.tile([128, TC * N], bf16)
                    m4 = m[:].rearrange("r (t p j) -> r t p j", t=TC, p=P)
                    x4 = X[:, t0 * N : (t0 + TC) * N].rearrange(
                        "r (t p j) -> r t p j", t=TC, p=P
                    )
                    vr = vrep[:].rearrange("r (p j) -> r p j", p=P).unsqueeze(1).broadcast_to([128, TC, P, J])
                    nc.vector.tensor_tensor(m4, x4, vr, alu.mult)
                    for h in (8, 4, 2, 1):
                        nc.vector.tensor_tensor(
                            m4[:, :, 0:h, :], m4[:, :, 0:h, :],
                            m4[:, :, h : 2 * h, :], alu.add,
                        )
                    dst = blog[:, t0 * J : (t0 + TC) * J].rearrange(
                        "r (t j) -> r t j", t=TC
                    )
                    if first:
                        nc.vector.tensor_copy(dst, m4[:, :, 0, :])
                    else:
                        nc.vector.tensor_tensor(dst, dst, m4[:, :, 0, :], alu.add)

            def softmax_j():
                """cbuf = softmax_j(blog).  Logits are O(1e-2): no max-sub."""
                nc.scalar.activation(cbuf[:], blog[:], act.Exp)
                e3 = cbuf[:].rearrange("r (t j) -> r t j", t=T)
                ssum = small.tile([128, T], f32, tag="ssum")
                nc.vector.tensor_reduce(ssum[:], e3, mybir.AxisListType.X, alu.add)
                rcp = small.tile([128, T], f32, tag="rcp")
                nc.vector.reciprocal(rcp[:], ssum[:])
                rb = rcp[:].unsqueeze(2).broadcast_to([128, T, J])
                nc.vector.tensor_tensor(e3, e3, rb, alu.mult)

            def s_phase(s_ps):
                """s_ps = sum_k c * x_hat via Pi product + selector matmul."""
                for ch in range(NCH):
                    t0 = ch * TC
                    m = mpool.tile([128, TC * N], bf16)
                    m4 = m[:].rearrange("r (t p j) -> r t p j", t=TC, p=P)
                    x4 = X[:, t0 * N : (t0 + TC) * N].rearrange(
                        "r (t p j) -> r t p j", t=TC, p=P
                    )
                    cb = (
                        cbuf[:, t0 * J : (t0 + TC) * J]
                        .rearrange("r (t j) -> r t j", t=TC)
                        .unsqueeze(2)
                        .broadcast_to([128, TC, P, J])
                    )
                    nc.vector.tensor_tensor(m4, x4, cb, alu.mult)
                    for ti in range(TC):
                        t = t0 + ti
                        nc.tensor.matmul(
                            s_ps[:],
                            sel_sb[:],
                            m[:, ti * N : (ti + 1) * N],
                            start=(t == 0),
                            stop=(t == T - 1),
                        )

            # ---------------- routing ----------------
            v0, vrep0 = squash(s0_ps, 1.0 / J, True)
            db_phase(vrep0, first=True)
            softmax_j()
            s1_ps = ps_pool.tile([BL, N], f32, tag="s")
            s_phase(s1_ps)
            v1, vrep1 = squash(s1_ps, 1.0, True)
            db_phase(vrep1, first=False)
            softmax_j()
            s2_ps = ps_pool.tile([BL, N], f32, tag="s")
            s_phase(s2_ps)
            v2, _ = squash(s2_ps, 1.0, False)
            nc.sync.dma_start(out_d.ap(), v2[:])

    nc.compile()
    return nc


def kernel(x, weight):
    from concourse.bass_utils import run_bass_kernel_spmd

    key = "prog"
    if key not in _CACHE:
        _CACHE[key] = _build_program()
    nc = _CACHE[key]

    w_re, xbds, sel = _prep(x, weight)
    in_maps = [
        {"w_re": w_re, "xbd": xbds[c], "sel": sel} for c in range(NC_N)
    ]
    res = run_bass_kernel_spmd(nc, in_maps, list(range(NC_N)))
    outs = []
    for c in range(NC_N):
        o = res.results[c]["out"]  # [BL, N] in (p, j) layout
        outs.append(o.reshape(BL, P, J).transpose(0, 2, 1))
    return np.ascontiguousarray(np.concatenate(outs, axis=0).astype(np.float32))


# revision 10
# speedup vs baseline: 1.2710x; 1.2710x over previous
"""DenseCapsule dynamic-routing kernel for 8 Trainium2 NeuronCores.

Problem (per reference):
  x      [B=64, K=2048, Q=8]   fp32
  weight [J=32, K=2048, P=16, Q=8] fp32
  x_hat[b,j,k,p] = sum_q W[j,k,p,q] x[b,k,q]
  3 routing iterations (softmax over j, squash over p)
  out [B, J, P]

Sharding: data-parallel over batch (8 batches/core), weight replicated.

Per-core kernel strategy (memory regime):
  - x_hat (33.5MB fp32 for 8 batches) is NEVER written to HBM. It is
    computed once on the PE and kept SBUF-resident in bf16 (16.8MB).
  - Phase 1 matmul uses a block-diagonal x trick so the tiny q=8
    contraction still runs at full PE rate:
      lhsT = xbd[t]  [(k16,q8)=128, (k16,b8)=128]  (block-diag, host-built)
      rhs  = W_re[t] [(k16,q8)=128, (p16,j32)=512]
      out  = psum    [(k16,b8)=128, (p16,j32)=512] = x_hat tile
  - s[b,j,p] = sum_k c*x_hat runs on PE as a selector matmul: contract
    the (k16,b8) partition dim against a constant delta_b selector.
  - db[b,j,k] = sum_p v*x_hat runs on DVE: 4x-mode multiply then an
    in-place fold-tree over p (innermost j stays stride-1 = packed).
  - softmax over j: ACT exp + DVE reduces, j innermost.
"""

import numpy as np
import ml_dtypes

B, K, Q, J, P = 64, 2048, 8, 32, 16
NC_N = 8          # cores
BL = B // NC_N    # local batch = 8
KT = 16           # k's per tile
T = K // KT       # 128 tiles
N = P * J         # 512 free (p,j) layout: idx = p*32 + j
TC = 16           # k-tiles per routing chunk
NCH = T // TC     # 8 chunks

# weight dtype for the phase-1 matmul: "bf16" (now fp16) halves the dominant HBM
# stream (33.5MB -> 16.8MB); "fp32r" keeps fp32 inputs at full PE rate.
WDT = "bf16"

_CACHE = {}


def _prep(x, weight):
    x = np.ascontiguousarray(np.asarray(x, dtype=np.float32))
    weight = np.ascontiguousarray(np.asarray(weight, dtype=np.float32))
    np_wdt = np.float16 if WDT == "bf16" else np.float32

    # W_re[ks*8+q, t, p*32+j] = W[j, t*16+ks, p, q]  (contiguous along (t,pj)
    # per partition row so a 4-tile DMA is one 4KB-run descriptor per row)
    w5 = weight.reshape(J, T, KT, P, Q)
    w_re = np.ascontiguousarray(
        w5.transpose(2, 4, 1, 3, 0).reshape(KT * Q, T, N).astype(np_wdt)
    )

    # xbd per core: [ks*8+q, t, ks*8+b] = x[b, t*16+ks, q]
    xbds = []
    for c in range(NC_N):
        xc = x[c * BL : (c + 1) * BL]                      # [8, K, Q]
        xr = xc.reshape(BL, T, KT, Q).transpose(2, 3, 1, 0)  # [ks, q, t, b]
        z = np.zeros((KT, Q, T, KT, BL), dtype=np_wdt)
        for ks in range(KT):
            z[ks, :, :, ks, :] = xr[ks]
        xbds.append(z.reshape(KT * Q, T, KT * BL))

    # selector: sel8[ks*8+b, b'] = (b == b')
    sel = np.zeros((KT * BL, BL), dtype=np.float16)
    for ks in range(KT):
        for b in range(BL):
            sel[ks * BL + b, b] = 1.0
    return w_re, xbds, sel


def _build_program():
    import concourse.bass as bass
    import concourse.tile as tile
    import concourse.mybir as mybir
    from concourse import bacc

    f32 = mybir.dt.float32
    bf16 = mybir.dt.float16
    wdt = bf16 if WDT == "bf16" else mybir.dt.float32r
    alu = mybir.AluOpType
    act = mybir.ActivationFunctionType

    nc = bacc.Bacc("TRN2", target_bir_lowering=False, debug=False)

    w_d = nc.dram_tensor("w_re", [KT * Q, T, N], wdt, kind="ExternalInput")
    xbd_d = nc.dram_tensor("xbd", [KT * Q, T, KT * BL], wdt, kind="ExternalInput")
    vtmp_d = nc.dram_tensor("vtmp", [BL, N], bf16)
    sel_d = nc.dram_tensor("sel", [KT * BL, BL], bf16, kind="ExternalInput")
    out_d = nc.dram_tensor("out", [BL, N], f32, kind="ExternalOutput")

    with tile.TileContext(nc) as tc:
        with (
            tc.tile_pool(name="xhat", bufs=1) as xhat_pool,
            tc.tile_pool(name="wp", bufs=2) as wp,
            tc.tile_pool(name="xbp", bufs=2) as xbp,
            tc.tile_pool(name="selp", bufs=1) as selp,
            tc.tile_pool(name="mbuf", bufs=2) as mpool,
            tc.tile_pool(name="blogp", bufs=1) as blogp,
            tc.tile_pool(name="cbufp", bufs=1) as cbufp,
            tc.tile_pool(name="small", bufs=1) as small,
            tc.tile_pool(name="vrepp", bufs=2) as vrepp,
            tc.tile_pool(name="ph", bufs=5, space="PSUM") as ph_pool,
            tc.tile_pool(name="ps", bufs=3, space="PSUM") as ps_pool,
        ):
            # persistent tensors
            X = xhat_pool.tile([128, T * N], bf16)       # x_hat, bf16
            sel_sb = selp.tile([128, BL], bf16)
            nc.sync.dma_start(sel_sb[:], sel_d.ap())
            blog = blogp.tile([128, T * J], bf16)        # b_logits [(k,b),(t,j)]
            cbuf = cbufp.tile([128, T * J], bf16)        # exp / c   [(k,b),(t,j)]

            # ---------------- phase 1: x_hat + s0 ----------------
            WB = 4    # W k-tiles per DMA
            XB = 8    # xbd k-tiles per DMA
            s0_ps = ps_pool.tile([BL, N], f32, tag="s")
            wts = {}
            xbs = {}
            for t in range(T):
                if t % WB == 0:
                    wt = wp.tile([128, WB * N], wdt)
                    nc.sync.dma_start(
                        wt[:], w_d.ap()[:, t : t + WB, :].rearrange("r t n -> r (t n)")
                    )
                    wts[t] = wt
                if t % XB == 0:
                    xb = xbp.tile([128, XB * KT * BL], wdt)
                    nc.sync.dma_start(
                        xb[:],
                        xbd_d.ap()[:, t : t + XB, :].rearrange("r t n -> r (t n)"),
                    )
                    xbs[t] = xb
                wt = wts[t - t % WB]
                xb = xbs[t - t % XB]
                ph = ph_pool.tile([128, N], f32)
                nc.tensor.matmul(
                    ph[:],
                    xb[:, (t % XB) * KT * BL : (t % XB + 1) * KT * BL],
                    wt[:, (t % WB) * N : (t % WB + 1) * N],
                    start=True,
                    stop=True,
                )
                # cast to bf16 into resident X; 2/3 ACT, 1/3 DVE
                if t % 3 != 2:
                    nc.scalar.copy(X[:, t * N : (t + 1) * N], ph[:])
                else:
                    nc.vector.tensor_copy(X[:, t * N : (t + 1) * N], ph[:])
                # s0 accumulation: contract (k16,b8) with delta_b selector
                nc.tensor.matmul(
                    s0_ps[:],
                    sel_sb[:],
                    X[:, t * N : (t + 1) * N],
                    start=(t == 0),
                    stop=(t == T - 1),
                )

            def squash(s_ps, scale, want_vrep):
                """v = squash(scale * s_ps) over p.  Returns (v fp32 [8,N],
                vrep bf16 [128,N] or None)."""
                s_s = small.tile([BL, N], f32, tag="s_s")
                nc.vector.tensor_scalar_mul(s_s[:], s_ps[:], scale)
                sq = small.tile([BL, N], f32, tag="sq")
                nc.vector.tensor_tensor(sq[:], s_s[:], s_s[:], alu.mult)
                # fold over p (p stride = J, j stride = 1)
                v3 = sq[:].rearrange("b (p j) -> b p j", p=P)
                for h in (8, 4, 2, 1):
                    nc.vector.tensor_tensor(
                        v3[:, 0:h, :], v3[:, 0:h, :], v3[:, h : 2 * h, :], alu.add
                    )
                n2 = small.tile([BL, J], f32, tag="n2")
                nc.vector.tensor_copy(n2[:], v3[:, 0, :])
                nrm = small.tile([BL, J], f32, tag="nrm")
                nc.scalar.sqrt(nrm[:], n2[:])
                den = small.tile([BL, J], f32, tag="den")
                nc.vector.tensor_scalar_add(den[:], n2[:], 1.0)
                rec = small.tile([BL, J], f32, tag="rec")
                nc.vector.reciprocal(rec[:], den[:])
                fct = small.tile([BL, J], f32, tag="fct")
                nc.vector.tensor_tensor(fct[:], nrm[:], rec[:], alu.mult)
                v = small.tile([BL, N], f32, tag="v")
                fb = fct[:].unsqueeze(1).broadcast_to([BL, P, J])
                nc.vector.tensor_tensor(
                    v[:].rearrange("b (p j) -> b p j", p=P),
                    s_s[:].rearrange("b (p j) -> b p j", p=P),
                    fb,
                    alu.mult,
                )
                vrep = None
                if want_vrep:
                    vbf = small.tile([BL, N], bf16, tag="vbf")
                    nc.vector.tensor_copy(vbf[:], v[:])
                    vrep = vrepp.tile([128, N], bf16)
                    for ks in range(KT):
                        nc.sync.dma_start(vrep[ks * BL : (ks + 1) * BL, :], vbf[:])
                return v, vrep

            def db_phase(vrep, first):
                """blog (+)= sum_p X * vrep, chunked over t."""
                for ch in range(NCH):
                    t0 = ch * TC
                    m = mpool.tile([128, TC * N], bf16)
                    m4 = m[:].rearrange("r (t p j) -> r t p j", t=TC, p=P)
                    x4 = X[:, t0 * N : (t0 + TC) * N].rearrange(
                        "r (t p j) -> r t p j", t=TC, p=P
                    )
                    vr = vrep[:].rearrange("r (p j) -> r p j", p=P).unsqueeze(1).broadcast_to([128, TC, P, J])
                    nc.vector.tensor_tensor(m4, x4, vr, alu.mult)
                    for h in (8, 4, 2, 1):
                        nc.vector.tensor_tensor(
                            m4[:, :, 0:h, :], m4[:, :, 0:h, :],
                            m4[:, :, h : 2 * h, :], alu.add,
                        )
                    dst = blog[:, t0 * J : (t0 + TC) * J].rearrange(
                        "r (t j) -> r t j", t=TC
                    )
                    if first:
                        nc.vector.tensor_copy(dst, m4[:, :, 0, :])
                    else:
                        nc.vector.tensor_tensor(dst, dst, m4[:, :, 0, :], alu.add)

            def softmax_s_phase(s_ps):
                """Chunked: softmax_j(blog) -> cbuf, Pi = c*X, s_ps += sel^T Pi.
                Softmax over j is local per (partition row, t), so it chunks.
                Logits are O(1e-2): no max-subtraction needed."""
                for ch in range(NCH):
                    t0 = ch * TC
                    bl3 = blog[:, t0 * J : (t0 + TC) * J].rearrange(
                        "r (t j) -> r t j", t=TC
                    )
                    e3 = cbuf[:, t0 * J : (t0 + TC) * J].rearrange(
                        "r (t j) -> r t j", t=TC
                    )
                    nc.scalar.activation(
                        e3.rearrange("r t j -> r (t j)"),
                        bl3.rearrange("r t j -> r (t j)"),
                        act.Exp,
                    )
                    ssum = small.tile([128, TC], f32, tag="ssum")
                    nc.vector.tensor_reduce(ssum[:], e3, mybir.AxisListType.X, alu.add)
                    rcp = small.tile([128, TC], f32, tag="rcp")
                    nc.vector.reciprocal(rcp[:], ssum[:])
                    rb = rcp[:].unsqueeze(2).broadcast_to([128, TC, J])
                    nc.vector.tensor_tensor(e3, e3, rb, alu.mult)
                    m = mpool.tile([128, TC * N], bf16)
                    m4 = m[:].rearrange("r (t p j) -> r t p j", t=TC, p=P)
                    x4 = X[:, t0 * N : (t0 + TC) * N].rearrange(
                        "r (t p j) -> r t p j", t=TC, p=P
                    )
                    cb = (
                        cbuf[:, t0 * J : (t0 + TC) * J]
                        .rearrange("r (t j) -> r t j", t=TC)
                        .unsqueeze(2)
                        .broadcast_to([128, TC, P, J])
                    )
                    nc.vector.tensor_tensor(m4, x4, cb, alu.mult)
                    for ti in range(TC):
                        t = t0 + ti
                        nc.tensor.matmul(
                            s_ps[:],
                            sel_sb[:],
                            m[:, ti * N : (ti + 1) * N],
                            start=(t == 0),
                            stop=(t == T - 1),
                        )

            # ---------------- routing ----------------
            v0, vrep0 = squash(s0_ps, 1.0 / J, True)
            db_phase(vrep0, first=True)
            s1_ps = ps_pool.tile([BL, N], f32, tag="s")
            softmax_s_phase(s1_ps)
            v1, vrep1 = squash(s1_ps, 1.0, True)
            db_phase(vrep1, first=False)
            s2_ps = ps_pool.tile([BL, N], f32, tag="s")
            softmax_s_phase(s2_ps)
            v2, _ = squash(s2_ps, 1.0, False)
            nc.sync.dma_start(out_d.ap(), v2[:])

    nc.compile()
    return nc


def kernel(x, weight):
    from concourse.bass_utils import run_bass_kernel_spmd

    key = "prog"
    if key not in _CACHE:
        _CACHE[key] = _build_program()
    nc = _CACHE[key]

    w_re, xbds, sel = _prep(x, weight)
    in_maps = [
        {"w_re": w_re, "xbd": xbds[c], "sel": sel} for c in range(NC_N)
    ]
    res = run_bass_kernel_spmd(nc, in_maps, list(range(NC_N)))
    outs = []
    for c in range(NC_N):
        o = res.results[c]["out"]  # [BL, N] in (p, j) layout
        outs.append(o.reshape(BL, P, J).transpose(0, 2, 1))
    return np.ascontiguousarray(np.concatenate(outs, axis=0).astype(np.float32))
